# revision 6
# baseline (speedup 1.0000x reference)
"""Trainium2 Bass kernel for nn_Cont_InfoNCE (pairwise max cross-correlation + CE loss).

Math: the reference's irfft(F1[i] * conj(F2[j]) / power) is the linear
cross-correlation of the centered rows at every lag, scaled by the positive
constant 1/(power*(T-1)).  max over lags therefore commutes with the scaling,
so dist[i,j] = max_l sum_t f1c[i,t] * f2c[j,t+l] / (1023*s1[i]*s2[j]).

We compute the correlation for all 2047 lags as dense bf16 matmuls on the
tensor engine (fp32 PSUM accumulation), max-reduce over lags on the vector
engine, and do the row-wise CE on device.  Sharding: rows of zis across the 8
cores (32 rows each), zjs replicated; each core emits a partial loss scalar
and the host sums the 8 partials.

Tiling (per core; A = centered local zis rows (32,1024), B = centered zjs):
  Apad[i]   = [0^127, A[i], 0^129]                       (32, 1280) bf16
  Tau[t,i,u]= Apad[i, u+t]          (Hankel gather via DMA from DRAM)
  BT_c[t,j] = B[j, 128c+t]          (PE transposes)
  for lam in 0..15, jt in 0..1, ic in 0..7:
    psum[j,ii,d] = sum_{c: e=c-lam+8 in [0,8]} BT_c[:,jtile].T @ Tau[:,ic,128e+d]
  which equals C[i, j, l] at lag l = 128*lam - 897 - d' ... covering every lag
  in [-1024, 1023] exactly once (l = -1024 slot is identically 0, mirroring
  the reference's zero-overlap k=1024 slot).
"""

import sys

if "/opt/trn_rl_repo" not in sys.path:
    sys.path.insert(0, "/opt/trn_rl_repo")

from contextlib import ExitStack

import numpy as np

import concourse.bass as bass
import concourse.mybir as mybir
from concourse import bacc, tile
from concourse.bass_utils import run_bass_kernel_spmd
from concourse.masks import make_identity

F32 = mybir.dt.float32
BF16 = mybir.dt.bfloat16
I32 = mybir.dt.int32
X = mybir.AxisListType.X
ALU = mybir.AluOpType
ACT = mybir.ActivationFunctionType

M, T = 256, 1024
NCORES = 8
NLOC = M // NCORES  # 32 rows of zis per core
NIC = 4             # i-rows per i-chunk
NCHUNK = NLOC // NIC  # 8 i-chunks
TAU_U = 1152        # Hankel window extent: 128*8 + 128
APAD = 1280         # 127 zeros + 1024 + 129 zeros


def _rsqrt_scaled(nc, pool, out, ss, k, parts, tag):
    """out = sqrt(k / ss), elementwise on a (parts,1) fp32 column.

    vector.reciprocal (accurate iterative divide) + ACT Sqrt + one Newton
    step to wash out the Sqrt table's loose ULP budget.
    """
    a = pool.tile([parts, 1], F32, tag=tag + "_a")
    nc.vector.reciprocal(a, ss)
    v = pool.tile([parts, 1], F32, tag=tag + "_v")
    nc.vector.tensor_scalar_mul(v, a, float(k))
    y0 = pool.tile([parts, 1], F32, tag=tag + "_y0")
    nc.scalar.sqrt(y0, v)
    ry = pool.tile([parts, 1], F32, tag=tag + "_ry")
    nc.vector.reciprocal(ry, y0)
    t2 = pool.tile([parts, 1], F32, tag=tag + "_t2")
    # t2 = (v * 0.5) * (1/y0)
    nc.vector.scalar_tensor_tensor(t2, in0=v, scalar=0.5, in1=ry, op0=ALU.mult, op1=ALU.mult)
    # out = (y0 * 0.5) + t2
    nc.vector.scalar_tensor_tensor(out, in0=y0, scalar=0.5, in1=t2, op0=ALU.mult, op1=ALU.add)


def build_nc():
    nc = bacc.Bacc("TRN2", target_bir_lowering=False)
    zis_loc = nc.dram_tensor("zis_loc", [NLOC, T], F32, kind="ExternalInput")
    zjs_full = nc.dram_tensor("zjs_full", [M, T], F32, kind="ExternalInput")
    speeds_loc = nc.dram_tensor("speeds_loc", [NLOC, 1], I32, kind="ExternalInput")
    loss_part = nc.dram_tensor("loss_part", [1, 1], F32, kind="ExternalOutput")

    with tile.TileContext(nc) as tc, ExitStack() as ctx:
        consts = ctx.enter_context(tc.tile_pool(name="consts", bufs=1))
        prep = ctx.enter_context(tc.tile_pool(name="prep", bufs=2))
        dram = ctx.enter_context(tc.tile_pool(name="dram", bufs=1, space="DRAM"))
        taup = ctx.enter_context(tc.tile_pool(name="taup", bufs=3))
        ps_aux = ctx.enter_context(tc.tile_pool(name="ps_aux", bufs=2, space="PSUM"))
        ps_main = ctx.enter_context(tc.tile_pool(name="ps_main", bufs=6, space="PSUM"))

        # ---------------- constants ----------------
        ident_bf = consts.tile([128, 128], BF16)
        make_identity(nc, ident_bf)
        ident_f32 = consts.tile([128, 128], F32)
        make_identity(nc, ident_f32)
        ones_col = consts.tile([NLOC, 1], F32)
        nc.gpsimd.memset(ones_col, 1.0)
        jidx_i = consts.tile([NLOC, M], I32)
        nc.gpsimd.iota(jidx_i, [[1, M]], base=0, channel_multiplier=0)
        jidx_f = consts.tile([NLOC, M], F32)
        nc.vector.tensor_copy(jidx_f, jidx_i)
        sp_i = prep.tile([NLOC, 1], I32)
        nc.sync.dma_start(sp_i, speeds_loc[:, :])
        sp_f = prep.tile([NLOC, 1], F32)
        nc.vector.tensor_copy(sp_f, sp_i)

        # ---------------- A (local zis rows): center, stats, Apad -> DRAM ----
        a_in = prep.tile([NLOC, T], F32)
        nc.sync.dma_start(a_in, zis_loc[:, :])
        asum = prep.tile([NLOC, 1], F32)
        nc.vector.reduce_sum(asum, a_in, axis=X)
        amean = prep.tile([NLOC, 1], F32)
        nc.scalar.mul(amean, asum, 1.0 / T)
        junk_a = prep.tile([NLOC, T], F32)
        ssraw_a = prep.tile([NLOC, 1], F32)
        nc.scalar.activation(junk_a, a_in, ACT.Square, accum_out=ssraw_a)
        mu2a = prep.tile([NLOC, 1], F32)
        nc.scalar.activation(mu2a, amean, ACT.Square)
        ss1 = prep.tile([NLOC, 1], F32)
        # ss1 = ssraw - T*mean^2  (= sum of squares of centered row)
        nc.vector.scalar_tensor_tensor(ss1, in0=mu2a, scalar=-float(T), in1=ssraw_a, op0=ALU.mult, op1=ALU.add)
        r1 = prep.tile([NLOC, 1], F32)
        _rsqrt_scaled(nc, prep, r1, ss1, 1.0 / (T - 1), NLOC, "r1")  # 1/((T-1)*s1)

        apad_sb = prep.tile([NLOC, APAD], BF16)
        nc.vector.memset(apad_sb, 0.0)
        nc.vector.tensor_scalar(apad_sb[:, 127:127 + T], a_in, amean, None, op0=ALU.subtract)
        apad_d = dram.tile([NLOC, APAD], BF16)
        nc.sync.dma_start(apad_d[:, :], apad_sb[:, :])

        # ---------------- B (all zjs rows): center -> bf16, stats ------------
        bc_tiles = []
        r2_tiles = []
        for jt in range(2):
            b_in = prep.tile([128, T], F32, tag="b_in")
            nc.sync.dma_start(b_in, zjs_full[jt * 128:(jt + 1) * 128, :])
            bsum = prep.tile([128, 1], F32, tag="bsum")
            nc.vector.reduce_sum(bsum, b_in, axis=X)
            bmean = prep.tile([128, 1], F32, tag="bmean")
            nc.scalar.mul(bmean, bsum, 1.0 / T)
            junk_b = prep.tile([128, T], F32, tag="junk_b")
            ssraw_b = prep.tile([128, 1], F32, tag="ssraw_b")
            nc.scalar.activation(junk_b, b_in, ACT.Square, accum_out=ssraw_b)
            mu2b = prep.tile([128, 1], F32, tag="mu2b")
            nc.scalar.activation(mu2b, bmean, ACT.Square)
            ss2 = prep.tile([128, 1], F32, tag="ss2")
            nc.vector.scalar_tensor_tensor(ss2, in0=mu2b, scalar=-float(T), in1=ssraw_b, op0=ALU.mult, op1=ALU.add)
            r2 = consts.tile([128, 1], F32, tag=f"r2_{jt}")
            _rsqrt_scaled(nc, prep, r2, ss2, float(T - 1), 128, "r2")  # 1/s2
            r2_tiles.append(r2)
            bc = consts.tile([128, T], BF16, tag=f"bc_{jt}")
            nc.vector.tensor_scalar(bc, b_in, bmean, None, op0=ALU.subtract)
            bc_tiles.append(bc)

        # ---------------- BT[t, c, j] = B[j, 128c+t] via PE transposes -------
        bt = consts.tile([128, 8, M], BF16)
        for jt in range(2):
            for c in range(8):
                ps_t = ps_aux.tile([128, 128], BF16, tag="aux")
                nc.tensor.transpose(ps_t, bc_tiles[jt][:, 128 * c:128 * (c + 1)], ident_bf)
                nc.scalar.copy(bt[:, c, jt * 128:(jt + 1) * 128], ps_t)

        # ---------------- main correlation loop ------------------------------
        cmax_p = [
            consts.tile([128, 16, NLOC], F32, tag=f"cmax_{jt}", name=f"cmax_{jt}")
            for jt in range(2)
        ]
        for ic in range(NCHUNK):
            tau = taup.tile([128, NIC, TAU_U], BF16, tag="tau")
            src = apad_d[NIC * ic:NIC * (ic + 1), 0:TAU_U]
            v = src.unsqueeze(0).broadcast_to((128, NIC, TAU_U))
            lst = v.ap
            lst[0] = [1, 128]  # Hankel: dest partition t reads Apad at +t elements
            v.ap = lst
            nc.sync.dma_start(tau[:, :, :], v)
            for jt in range(2):
                for lam in range(16):
                    cs = [c for c in range(8) if 0 <= c - lam + 8 <= 8]
                    ps = ps_main.tile([128, NIC, 128], F32, tag="grp")
                    for k, c in enumerate(cs):
                        e = c - lam + 8
                        nc.tensor.matmul(
                            ps[:, :, :],
                            lhsT=bt[:, c, jt * 128:(jt + 1) * 128],
                            rhs=tau[:, :, 128 * e:128 * (e + 1)],
                            start=(k == 0),
                            stop=(k == len(cs) - 1),
                        )
                    nc.vector.reduce_max(cmax_p[jt][:, lam, NIC * ic:NIC * (ic + 1)], ps[:, :, :], axis=X)

        # ---------------- normalize + transpose to (i, j) ---------------------
        dist_t = prep.tile([NLOC, M], F32)
        for jt in range(2):
            cm2 = prep.tile([128, NLOC], F32, tag="cm2")
            nc.vector.reduce_max(cm2, cmax_p[jt].rearrange("p l i -> p i l"), axis=X)
            cms = prep.tile([128, NLOC], F32, tag="cms")
            nc.vector.tensor_scalar(cms, cm2, r2_tiles[jt], None, op0=ALU.mult)
            ps_d = ps_aux.tile([NLOC, 128], F32, tag="aux")
            nc.tensor.transpose(ps_d, cms, ident_f32)
            nc.vector.tensor_scalar(dist_t[:, jt * 128:(jt + 1) * 128], ps_d, r1, None, op0=ALU.mult)

        # ---------------- cross-entropy (sum over local rows) -----------------
        mrow = prep.tile([NLOC, 1], F32)
        nc.vector.reduce_max(mrow, dist_t, axis=X)
        negm = prep.tile([NLOC, 1], F32)
        nc.vector.tensor_scalar_mul(negm, mrow, -1.0)
        expj = prep.tile([NLOC, M], F32)
        sumexp = prep.tile([NLOC, 1], F32)
        nc.scalar.activation(expj, dist_t, ACT.Exp, bias=negm, accum_out=sumexp)
        lse = prep.tile([NLOC, 1], F32)
        nc.scalar.activation(lse, sumexp, ACT.Ln)
        onehot = prep.tile([NLOC, M], F32)
        nc.vector.tensor_scalar(onehot, jidx_f, sp_f, None, op0=ALU.is_equal)
        junk_p = prep.tile([NLOC, M], F32)
        picked = prep.tile([NLOC, 1], F32)
        nc.vector.scalar_tensor_tensor(
            junk_p, in0=dist_t, scalar=1.0, in1=onehot, op0=ALU.mult, op1=ALU.mult, accum_out=picked
        )
        term = prep.tile([NLOC, 1], F32)
        nc.vector.tensor_add(term, lse, mrow)
        term2 = prep.tile([NLOC, 1], F32)
        nc.vector.tensor_sub(term2, term, picked)
        ps_l = ps_aux.tile([1, 1], F32, tag="aux")
        nc.tensor.matmul(ps_l, lhsT=term2, rhs=ones_col, start=True, stop=True)
        lsb = prep.tile([1, 1], F32)
        nc.vector.tensor_copy(lsb, ps_l)
        nc.sync.dma_start(loss_part[:, :], lsb)

    nc.finalize()
    return nc


_NC_CACHE = None
LAST_RESULT = None


def run(zis, zjs, speeds, trace=False):
    global _NC_CACHE, LAST_RESULT
    if _NC_CACHE is None:
        _NC_CACHE = build_nc()
    zis = np.ascontiguousarray(np.asarray(zis), dtype=np.float32)
    zjs = np.ascontiguousarray(np.asarray(zjs), dtype=np.float32)
    sp = np.asarray(speeds).astype(np.int32).reshape(M, 1)
    in_maps = [
        {
            "zis_loc": np.ascontiguousarray(zis[c * NLOC:(c + 1) * NLOC]),
            "zjs_full": zjs,
            "speeds_loc": np.ascontiguousarray(sp[c * NLOC:(c + 1) * NLOC]),
        }
        for c in range(NCORES)
    ]
    res = run_bass_kernel_spmd(_NC_CACHE, in_maps, core_ids=list(range(NCORES)), trace=trace)
    LAST_RESULT = res
    total = sum(float(r["loss_part"][0, 0]) for r in res.results)
    return np.float32(total)


def kernel(zis, zjs, speeds):
    return run(zis, zjs, speeds, trace=False)
